# revision 6
# baseline (speedup 1.0000x reference)
"""Self-contained Trainium2 Bass kernel for the bidirectional-LSTM decoder
(nn_Decoder): 2-direction LSTM scan over T=100 steps with a fixed input,
followed by a 32000-way vocab projection and log_softmax, on 8 NeuronCores.

v2 distribution (single fused SPMD launch):
  - Scan: dir x batch-quarter grid. Core c = (d=c//4, q=c%4) runs direction
    d's recurrence for batch rows 16q..16q+15. Per-step PE work is 4x less
    than a direction-redundant scan (moving dim 16 instead of 64). W_hh for
    a direction is assembled on-device from quarter shards via an AllGather
    inside each direction group (cuts input staging 4x).
  - gx (the x contribution to the gates, constant across steps) is folded
    on the host into a tiny [128,32,16] per-core constant - W_ih never
    touches the device.
  - Gate layout reordered host-side to [i,f,o,g] so each nonlinearity is a
    single strided activation instruction; the scan is pipelined in two
    half-H groups so DVE/ACT tails overlap the next group's matmuls.
  - LSTM outputs are AllGathered across all 8 cores in 5 chunks of 20
    steps, overlapped with the scan.
  - fc + log_softmax: vocab-split (4000 rows/core), single pass. Logits
    (bias included) are written to a bf16 DRAM stash while the Activation
    engine accumulates exp-sums; the softmax normalizer is AllReduced in 5
    groups of 10 token tiles so normalization (pass 2: stash + (-logZ) on
    the ACT engine) overlaps pass-1 matmuls of the next group.
  - All matmuls bf16 (fp32 PSUM accumulation); cell state c stays fp32.
"""

import sys

if "/opt/trn_rl_repo" not in sys.path:
    sys.path.insert(0, "/opt/trn_rl_repo")

from contextlib import ExitStack

import numpy as np

import concourse.bass as bass
import concourse.tile as tile
from concourse import mybir
from concourse.bass_utils import run_bass_kernel_spmd

F32 = mybir.dt.float32
BF16 = mybir.dt.bfloat16
NP_BF16 = mybir.dt.np(BF16)
B = 64
H = 1024
V = 32000
NCORES = 8
NCH = 5  # outs-exchange / softmax-normalizer chunks

MAX_WAITS = 1


def split_multiwait(nc):
    """The walrus build in this environment rejects any instruction carrying
    more than one semaphore wait; hoist excess waits onto chained NOPs
    (sem-ge waits commute, so this preserves semantics)."""
    import bass_rust

    n_split = 0
    for f in nc.m.functions:
        for bb in f.blocks:
            new_insts = []
            changed = False
            for ins in bb.instructions:
                si = ins.sync_info
                if si is not None and si.on_wait and len(si.on_wait) > MAX_WAITS:
                    waits = list(si.on_wait)
                    extra, keep = waits[:-MAX_WAITS], waits[-MAX_WAITS:]
                    for j in range(0, len(extra), MAX_WAITS):
                        nop = bass_rust.InstNoOp(name=f"{ins.name}-wsplit{j}")
                        nop.engine = ins.engine
                        nop.sync_info = mybir.SyncInfo(
                            on_wait=extra[j : j + MAX_WAITS], on_update=[]
                        )
                        new_insts.append(nop)
                        n_split += 1
                    ins.sync_info = mybir.SyncInfo(
                        on_wait=keep, on_update=list(si.on_update)
                    )
                    changed = True
                new_insts.append(ins)
            if changed:
                bb.instructions = new_insts
    return n_split


def build_v2(T, v_loc=V // NCORES, timing=False, reps=1):
    n_tok = B * T
    n_tt = n_tok // 128
    spc = T // NCH  # steps per chunk
    tpg = n_tt // NCH  # token tiles per normalizer group
    assert T % (2 * NCH) == 0 and n_tok % 128 == 0 and v_loc % 500 == 0

    nc = bass.Bass(num_devices=NCORES)
    whh_q = nc.declare_dram_parameter("whh_q", [2, 128, 32, 128], BF16, isOutput=False)
    gxT = nc.declare_dram_parameter("gxT", [128, 32, 16], F32, isOutput=False)
    h0T = nc.declare_dram_parameter("h0T", [128, 8, 16], BF16, isOutput=False)
    c0T = nc.declare_dram_parameter("c0T", [128, 8, 16], F32, isOutput=False)
    fcwT = nc.declare_dram_parameter("fcwT", [128, 16, v_loc], BF16, isOutput=False)
    fcb = nc.declare_dram_parameter("fcb", [1, v_loc], F32, isOutput=False)
    if timing:
        out = nc.dram_tensor("out", [n_tt, 128, v_loc], F32)
        chk = nc.declare_dram_parameter("chk", [128, 64], F32, isOutput=True)
    else:
        out = nc.declare_dram_parameter("out", [n_tt, 128, v_loc], F32, isOutput=True)

    ci_whh = nc.dram_tensor("ci_whh", [2, 128, 32, 128], BF16)
    co_whh = nc.dram_tensor("co_whh", [8, 128, 32, 128], BF16)
    outs_c = [
        nc.dram_tensor(f"outs_c{k}", [spc // 2, 128, 2, 128], BF16) for k in range(NCH)
    ]
    outsall_c = [
        nc.dram_tensor(
            f"outsall_c{k}", [8, spc // 2, 128, 2, 128], BF16, addr_space="Shared"
        )
        for k in range(NCH)
    ]
    lgst = nc.dram_tensor("lgst", [n_tt, 128, v_loc], BF16)
    ccs_in = [nc.dram_tensor(f"ccs_in{g}", [128, tpg], F32) for g in range(NCH)]
    ccs_out = [
        nc.dram_tensor(f"ccs_out{g}", [128, tpg], F32, addr_space="Shared")
        for g in range(NCH)
    ]

    ACT = mybir.ActivationFunctionType
    DIR_GROUPS = [[0, 1, 2, 3], [4, 5, 6, 7]]
    ALL_GROUP = [list(range(NCORES))]

    with tile.TileContext(nc) as tc, ExitStack() as octx:
        for rep in range(reps):
            _emit_rep(
                nc, tc, rep, T, v_loc, n_tt, spc, tpg, timing,
                whh_q, gxT, h0T, c0T, fcwT, fcb, out,
                chk if timing else None,
                ci_whh, co_whh, outs_c, outsall_c, lgst, ccs_in, ccs_out,
                ACT, DIR_GROUPS, ALL_GROUP,
            )

    split_multiwait(nc)
    return nc


def _emit_rep(
    nc, tc, rep, T, v_loc, n_tt, spc, tpg, timing,
    whh_q, gxT, h0T, c0T, fcwT, fcb, out, chk,
    ci_whh, co_whh, outs_c, outsall_c, lgst, ccs_in, ccs_out,
    ACT, DIR_GROUPS, ALL_GROUP,
):
    with ExitStack() as ctx:
        # ---- W_hh gather (param -> internal -> AllGather within dir group) --
        nc.sync.dma_start(out=ci_whh[:], in_=whh_q[:])
        nc.gpsimd.collective_compute(
            "AllGather", mybir.AluOpType.bypass, replica_groups=DIR_GROUPS,
            ins=[ci_whh[:]], outs=[co_whh[:]],
        )

        sm = ctx.enter_context(tc.tile_pool(name=f"sm{rep}", bufs=1))
        gx_sb = sm.tile([128, 32, 16], F32)
        nc.sync.dma_start(out=gx_sb, in_=gxT[:])

        # fc weight low half: DMA issued early so it overlaps the scan.
        fcwlo_pool = ctx.enter_context(tc.tile_pool(name=f"fcwlo{rep}", bufs=1))
        w_lo = fcwlo_pool.tile([128, 16, v_loc // 2], BF16)
        nc.sync.dma_start(out=w_lo, in_=fcwT[:, :, 0 : v_loc // 2])

        # ---- scan phase ----
        with (
            tc.tile_pool(name=f"whh{rep}", bufs=1) as whh_pool,
            tc.tile_pool(name=f"hp{rep}", bufs=2) as hp,
            tc.tile_pool(name=f"cp{rep}", bufs=2) as cp,
            tc.tile_pool(name=f"wk{rep}", bufs=2) as wk,
            tc.tile_pool(name=f"ps{rep}", bufs=2, space="PSUM") as psum_scan,
        ):
            whh_sb = whh_pool.tile([128, 8, 32, 128], BF16)
            for j in range(8):
                nc.sync.dma_start(out=whh_sb[:, j, :, :], in_=co_whh[j])

            h_prev = hp.tile([128, 8, 16], BF16, tag="h")
            nc.sync.dma_start(out=h_prev, in_=h0T[:])
            c_prev = cp.tile([128, 8, 16], F32, tag="c")
            nc.sync.dma_start(out=c_prev, in_=c0T[:])

            for t in range(T):
                pg = psum_scan.tile([128, 32, 16], F32, tag="pg")
                gates = wk.tile([128, 32, 16], F32, tag="gates")
                nl = wk.tile([128, 32, 16], F32, tag="nl")
                h_new = hp.tile([128, 8, 16], BF16, tag="h")
                c_new = cp.tile([128, 8, 16], F32, tag="c")
                g4 = lambda tile_, u0: tile_.rearrange(
                    "p (s u) b -> p s u b", s=4
                )[:, :, u0 : u0 + 4, :]
                for u0 in (0, 4):
                    # each PSUM slice's start->stop accumulation chain must be
                    # contiguous on the PE: interleaving groups across slices
                    # produces wrong results on hardware.
                    for s in range(4):
                        for u in range(u0, u0 + 4):
                            m = s * 8 + u
                            for j in range(8):
                                nc.tensor.matmul(
                                    pg[:, m, :], whh_sb[:, j, m, :], h_prev[:, j, :],
                                    start=(j == 0), stop=(j == 7),
                                )
                    nc.vector.tensor_add(g4(gates, u0), g4(pg, u0), g4(gx_sb, u0))
                    gv = gates.rearrange("p (s u) b -> p s u b", s=4)
                    nv = nl.rearrange("p (s u) b -> p s u b", s=4)
                    nc.scalar.activation(
                        nv[:, 0:3, u0 : u0 + 4, :], gv[:, 0:3, u0 : u0 + 4, :],
                        ACT.Sigmoid,
                    )
                    nc.scalar.activation(
                        nv[:, 3, u0 : u0 + 4, :], gv[:, 3, u0 : u0 + 4, :], ACT.Tanh
                    )
                    t1 = wk.tile([128, 4, 16], F32, tag=f"t1_{u0}")
                    t2 = wk.tile([128, 4, 16], F32, tag=f"t2_{u0}")
                    tnc = wk.tile([128, 4, 16], F32, tag=f"tnc_{u0}")
                    cs = slice(u0, u0 + 4)
                    nc.vector.tensor_mul(t1, nv[:, 0, cs, :], nv[:, 3, cs, :])
                    nc.vector.tensor_mul(t2, nv[:, 1, cs, :], c_prev[:, cs, :])
                    nc.vector.tensor_add(c_new[:, cs, :], t1, t2)
                    nc.scalar.activation(tnc, c_new[:, cs, :], ACT.Tanh)
                    for jj in range(u0, u0 + 4):
                        nc.vector.tensor_mul(
                            h_new[:, jj, :], nv[:, 2, jj, :], tnc[:, jj - u0, :]
                        )
                k, tin = t // spc, t % spc
                nc.sync.dma_start(
                    out=outs_c[k][tin // 2][:, tin % 2, :],
                    in_=h_new.rearrange("p j b -> p (j b)"),
                )
                if tin == spc - 1:
                    nc.gpsimd.collective_compute(
                        "AllGather", mybir.AluOpType.bypass,
                        replica_groups=ALL_GROUP,
                        ins=[outs_c[k][:]], outs=[outsall_c[k][:]],
                    )
                h_prev, c_prev = h_new, c_new

        # ---- fc phase ----
        fc_sing = ctx.enter_context(tc.tile_pool(name=f"fcs{rep}", bufs=1))
        w_hi = fc_sing.tile([128, 16, v_loc // 2], BF16)
        nc.sync.dma_start(out=w_hi, in_=fcwT[:, :, v_loc // 2 : v_loc])
        bias_bc = fc_sing.tile([128, v_loc], F32)
        fcb_ap = fcb[:]
        nc.sync.dma_start(
            out=bias_bc,
            in_=bass.AP(
                tensor=fcb_ap.tensor, offset=fcb_ap.offset, ap=[[0, 128], [1, v_loc]]
            ),
        )
        slab = ctx.enter_context(tc.tile_pool(name=f"slab{rep}", bufs=2))
        wkf = ctx.enter_context(tc.tile_pool(name=f"wkf{rep}", bufs=2))
        psum_fc = ctx.enter_context(
            tc.tile_pool(name=f"psf{rep}", bufs=6, space="PSUM")
        )
        p2p = ctx.enter_context(tc.tile_pool(name=f"p2p{rep}", bufs=2))
        spool = ctx.enter_context(tc.tile_pool(name=f"sp{rep}", bufs=1))

        s_all = spool.tile([128, n_tt], F32)
        nc.vector.memset(s_all, 0.0)
        nlz = spool.tile([128, n_tt], F32)
        n_vc = v_loc // 500

        def pass1_tile(tt):
            k, tp = tt // (spc // 2), tt % (spc // 2)
            src = outsall_c[k][:]
            sl = slab.tile([128, 8, 2, 128], BF16, tag="sl")
            stride_s = (spc // 2) * 128 * 2 * 128
            nc.sync.dma_start(
                out=sl,
                in_=bass.AP(
                    tensor=src.tensor,
                    offset=src.offset + tp * (128 * 2 * 128),
                    ap=[[256, 128], [stride_s, 8], [128, 2], [1, 128]],
                ),
            )
            o_t = slab.tile([128, 16, 128], BF16, tag="o_t")
            for s in range(8):
                d_, q_ = s // 4, s % 4
                dst = o_t[:, d_ * 8 : d_ * 8 + 8, :].rearrange(
                    "p j (t b) -> p j t b", t=2
                )[:, :, :, q_ * 16 : q_ * 16 + 16]
                nc.vector.tensor_copy(
                    dst, sl[:, s].rearrange("p t (j b) -> p j t b", j=8)
                )
            lg = wkf.tile([128, v_loc], BF16, tag="lg")
            for c8 in range(n_vc):
                vs = slice(c8 * 500, (c8 + 1) * 500)
                wsb = w_lo if c8 < n_vc // 2 else w_hi
                lvs = slice(
                    (c8 % (n_vc // 2)) * 500, (c8 % (n_vc // 2)) * 500 + 500
                )
                ps = psum_fc.tile([128, 500], F32)
                for kt in range(16):
                    nc.tensor.matmul(
                        ps, o_t[:, kt, :], wsb[:, kt, lvs],
                        start=(kt == 0), stop=(kt == 15),
                    )
                nc.vector.tensor_add(lg[:, vs], ps, bias_bc[:, vs])
                ex = wkf.tile([128, 500], F32, tag="ex")
                part = wkf.tile([128, 1], F32, tag="part")
                nc.scalar.activation(ex, lg[:, vs], ACT.Exp, accum_out=part)
                nc.vector.tensor_add(
                    s_all[:, tt : tt + 1], s_all[:, tt : tt + 1], part
                )
            nc.sync.dma_start(out=lgst[tt], in_=lg)

        def reduce_group(g):
            t0 = g * tpg
            nc.gpsimd.dma_start(out=ccs_in[g][:], in_=s_all[:, t0 : t0 + tpg])
            nc.gpsimd.collective_compute(
                "AllReduce", mybir.AluOpType.add, replica_groups=ALL_GROUP,
                ins=[ccs_in[g][:]], outs=[ccs_out[g][:]],
            )
            s_glob = spool.tile([128, tpg], F32, tag=f"sg{g}")
            nc.gpsimd.dma_start(out=s_glob, in_=ccs_out[g][:])
            nc.scalar.activation(nlz[:, t0 : t0 + tpg], s_glob, ACT.Ln)
            nc.vector.tensor_scalar_mul(
                nlz[:, t0 : t0 + tpg], nlz[:, t0 : t0 + tpg], -1.0
            )

        def pass2_tile(tt):
            for hq in range(4):
                vs = slice(hq * (v_loc // 4), (hq + 1) * (v_loc // 4))
                st = p2p.tile([128, v_loc // 4], BF16, tag="st")
                nc.sync.dma_start(out=st, in_=lgst[tt][:, vs])
                ot = p2p.tile([128, v_loc // 4], F32, tag="ot")
                nc.scalar.activation(ot, st, ACT.Identity, bias=nlz[:, tt : tt + 1])
                nc.sync.dma_start(out=out[tt][:, vs], in_=ot)

        NG = n_tt // tpg
        for g in range(NG):
            for tt in range(g * tpg, (g + 1) * tpg):
                pass1_tile(tt)
            reduce_group(g)
            if g >= 1:
                for tt in range((g - 1) * tpg, g * tpg):
                    pass2_tile(tt)
        for tt in range((NG - 1) * tpg, NG * tpg):
            pass2_tile(tt)

        if timing:
            chk_sb = spool.tile([128, 64], F32)
            nc.vector.tensor_copy(chk_sb[:, :n_tt], nlz)
            nc.sync.dma_start(out=chk[:, :n_tt], in_=chk_sb[:, :n_tt])


def prep_v2(x, h0, c0, W_ih, W_hh, b_ih, b_hh, fc_W, fc_b, T):
    """Per-core in_maps. Core c = (d=c//4, q=c%4): direction d, batch rows
    16q..16q+15, W_hh k-chunk pair {2q, 2q+1}, vocab slice c."""
    v_loc = V // NCORES
    # gate reorder i,f,g,o -> i,f,o,g
    perm = np.concatenate(
        [np.arange(0, 2 * H), np.arange(3 * H, 4 * H), np.arange(2 * H, 3 * H)]
    )
    maps = []
    per_dir = {}
    for d in (0, 1):
        whh_p = W_hh[d][perm]
        per_dir[d] = {
            "whh": whh_p.reshape(32, 128, 8, 128).transpose(3, 2, 0, 1),
            # gx = x @ W_ih^T + b_ih + b_hh, permuted: [64, 4096]
            "gx": (x @ W_ih[d].T + b_ih[d] + b_hh[d])[:, perm],
        }
    for c in range(NCORES):
        d, q = c // 4, c % 4
        pd = per_dir[d]
        whh_qv = np.ascontiguousarray(
            pd["whh"][:, 2 * q : 2 * q + 2].transpose(1, 0, 2, 3)
        ).astype(NP_BF16)
        bs = slice(16 * q, 16 * q + 16)
        gxT = np.ascontiguousarray(
            pd["gx"][bs].reshape(16, 32, 128).transpose(2, 1, 0)
        ).astype(np.float32)
        h0T = np.ascontiguousarray(
            h0[d][bs].reshape(16, 8, 128).transpose(2, 1, 0)
        ).astype(NP_BF16)
        c0T = np.ascontiguousarray(
            c0[d][bs].reshape(16, 8, 128).transpose(2, 1, 0)
        ).astype(np.float32)
        wv = fc_W[c * v_loc : (c + 1) * v_loc]
        fcwT = np.ascontiguousarray(
            wv.reshape(v_loc, 16, 128).transpose(2, 1, 0)
        ).astype(NP_BF16)
        maps.append(
            {
                "whh_q": whh_qv,
                "gxT": gxT,
                "h0T": h0T,
                "c0T": c0T,
                "fcwT": fcwT,
                "fcb": np.ascontiguousarray(
                    fc_b[c * v_loc : (c + 1) * v_loc].reshape(1, v_loc)
                ).astype(np.float32),
            }
        )
    return maps


def assemble_output(results, T):
    """results[c]["out"] is [n_tt, 128, v_loc], token = t*64 + b."""
    v_loc = V // NCORES
    full = np.concatenate(
        [results[c]["out"].reshape(B * T, v_loc) for c in range(NCORES)], axis=1
    )  # [t*64+b, V]
    return np.ascontiguousarray(full.reshape(T, B, V).transpose(1, 0, 2))


_build_cache = {}


def kernel(x, h0, c0, W_ih, W_hh, b_ih, b_hh, fc_W, fc_b, max_len):
    T = int(max_len)
    x = np.asarray(x, np.float32)
    h0 = np.asarray(h0, np.float32)
    c0 = np.asarray(c0, np.float32)
    W_ih = np.asarray(W_ih, np.float32)
    W_hh = np.asarray(W_hh, np.float32)
    b_ih = np.asarray(b_ih, np.float32)
    b_hh = np.asarray(b_hh, np.float32)
    fc_W = np.asarray(fc_W, np.float32)
    fc_b = np.asarray(fc_b, np.float32)

    if T not in _build_cache:
        _build_cache[T] = build_v2(T)
    nc = _build_cache[T]
    maps = prep_v2(x, h0, c0, W_ih, W_hh, b_ih, b_hh, fc_W, fc_b, T)
    res = run_bass_kernel_spmd(nc, maps, core_ids=list(range(NCORES)))
    return assemble_output([res.results[c] for c in range(NCORES)], T)


# revision 11
# speedup vs baseline: 1.0769x; 1.0769x over previous
"""Self-contained Trainium2 Bass kernel for the bidirectional-LSTM decoder
(nn_Decoder): 2-direction LSTM scan over T=100 steps with a fixed input,
followed by a 32000-way vocab projection and log_softmax, on 8 NeuronCores.

v2 distribution (single fused SPMD launch):
  - Scan: dir x batch-quarter grid. Core c = (d=c//4, q=c%4) runs direction
    d's recurrence for batch rows 16q..16q+15. Per-step PE work is 4x less
    than a direction-redundant scan (moving dim 16 instead of 64). W_hh for
    a direction is assembled on-device from quarter shards via an AllGather
    inside each direction group (cuts input staging 4x).
  - gx (the x contribution to the gates, constant across steps) is folded
    on the host into a tiny [128,32,16] per-core constant - W_ih never
    touches the device.
  - Gate layout reordered host-side to [i,f,o,g] so each nonlinearity is a
    single strided activation instruction; the scan is pipelined in two
    half-H groups so DVE/ACT tails overlap the next group's matmuls.
  - LSTM outputs are AllGathered across all 8 cores in 5 chunks of 20
    steps, overlapped with the scan.
  - fc + log_softmax: vocab-split (4000 rows/core), single pass. Logits
    (bias included) are written to a bf16 DRAM stash while the Activation
    engine accumulates exp-sums; the softmax normalizer is AllReduced in 5
    groups of 10 token tiles so normalization (pass 2: stash + (-logZ) on
    the ACT engine) overlaps pass-1 matmuls of the next group.
  - All matmuls bf16 (fp32 PSUM accumulation); cell state c stays fp32.
"""

import sys

if "/opt/trn_rl_repo" not in sys.path:
    sys.path.insert(0, "/opt/trn_rl_repo")

from contextlib import ExitStack

import numpy as np

import concourse.bass as bass
import concourse.tile as tile
from concourse import mybir
from concourse.bass_utils import run_bass_kernel_spmd

F32 = mybir.dt.float32
BF16 = mybir.dt.bfloat16
FP8 = mybir.dt.float8e4
NP_BF16 = mybir.dt.np(BF16)
NP_FP8 = mybir.dt.np(FP8)
W8_SCALE = 512.0
B = 64
H = 1024
V = 32000
NCORES = 8
NCH = 5  # outs-exchange / softmax-normalizer chunks

MAX_WAITS = 1


def split_multiwait(nc):
    """The walrus build in this environment rejects any instruction carrying
    more than one semaphore wait; hoist excess waits onto chained NOPs
    (sem-ge waits commute, so this preserves semantics)."""
    import bass_rust

    n_split = 0
    for f in nc.m.functions:
        for bb in f.blocks:
            new_insts = []
            changed = False
            for ins in bb.instructions:
                si = ins.sync_info
                if si is not None and si.on_wait and len(si.on_wait) > MAX_WAITS:
                    waits = list(si.on_wait)
                    extra, keep = waits[:-MAX_WAITS], waits[-MAX_WAITS:]
                    for j in range(0, len(extra), MAX_WAITS):
                        nop = bass_rust.InstNoOp(name=f"{ins.name}-wsplit{j}")
                        nop.engine = ins.engine
                        nop.sync_info = mybir.SyncInfo(
                            on_wait=extra[j : j + MAX_WAITS], on_update=[]
                        )
                        new_insts.append(nop)
                        n_split += 1
                    ins.sync_info = mybir.SyncInfo(
                        on_wait=keep, on_update=list(si.on_update)
                    )
                    changed = True
                new_insts.append(ins)
            if changed:
                bb.instructions = new_insts
    return n_split


def build_v2(T, v_loc=V // NCORES, timing=False, reps=1):
    n_tok = B * T
    n_tt = n_tok // 128
    spc = T // NCH  # steps per chunk
    tpg = n_tt // NCH  # token tiles per normalizer group
    assert T % (2 * NCH) == 0 and n_tok % 128 == 0 and v_loc % 500 == 0

    nc = bass.Bass(num_devices=NCORES)
    whh_q = nc.declare_dram_parameter("whh_q", [2, 128, 32, 128], BF16, isOutput=False)
    gxT = nc.declare_dram_parameter("gxT", [128, 32, 16], F32, isOutput=False)
    h0T = nc.declare_dram_parameter("h0T", [128, 8, 16], BF16, isOutput=False)
    c0T = nc.declare_dram_parameter("c0T", [128, 8, 16], F32, isOutput=False)
    fcwT = nc.declare_dram_parameter("fcwT", [128, 16, v_loc], FP8, isOutput=False)
    fcb = nc.declare_dram_parameter("fcb", [1, v_loc], F32, isOutput=False)
    if timing:
        out = nc.dram_tensor("out", [n_tt, 128, v_loc], F32)
        chk = nc.declare_dram_parameter("chk", [128, 64], F32, isOutput=True)
    else:
        out = nc.declare_dram_parameter("out", [n_tt, 128, v_loc], F32, isOutput=True)

    ci_whh = nc.dram_tensor("ci_whh", [2, 128, 32, 128], BF16)
    co_whh = nc.dram_tensor("co_whh", [8, 128, 32, 128], BF16)
    outs_c = [
        nc.dram_tensor(f"outs_c{k}", [spc // 2, 128, 2, 128], BF16) for k in range(NCH)
    ]
    outsall_c = [
        nc.dram_tensor(
            f"outsall_c{k}", [8, spc // 2, 128, 2, 128], BF16, addr_space="Shared"
        )
        for k in range(NCH)
    ]
    lgst = nc.dram_tensor("lgst", [n_tt, 128, v_loc], BF16)
    ccs_in = [nc.dram_tensor(f"ccs_in{g}", [128, tpg], F32) for g in range(NCH)]
    ccs_out = [
        nc.dram_tensor(f"ccs_out{g}", [128, tpg], F32, addr_space="Shared")
        for g in range(NCH)
    ]

    ACT = mybir.ActivationFunctionType
    DIR_GROUPS = [[0, 1, 2, 3], [4, 5, 6, 7]]
    ALL_GROUP = [list(range(NCORES))]

    with tile.TileContext(nc) as tc, ExitStack() as octx:
        for rep in range(reps):
            _emit_rep(
                nc, tc, rep, T, v_loc, n_tt, spc, tpg, timing,
                whh_q, gxT, h0T, c0T, fcwT, fcb, out,
                chk if timing else None,
                ci_whh, co_whh, outs_c, outsall_c, lgst, ccs_in, ccs_out,
                ACT, DIR_GROUPS, ALL_GROUP,
            )

    split_multiwait(nc)
    return nc


def _emit_rep(
    nc, tc, rep, T, v_loc, n_tt, spc, tpg, timing,
    whh_q, gxT, h0T, c0T, fcwT, fcb, out, chk,
    ci_whh, co_whh, outs_c, outsall_c, lgst, ccs_in, ccs_out,
    ACT, DIR_GROUPS, ALL_GROUP,
):
    with ExitStack() as ctx:
        # ---- W_hh gather (param -> internal -> AllGather within dir group) --
        nc.sync.dma_start(out=ci_whh[:], in_=whh_q[:])
        nc.gpsimd.collective_compute(
            "AllGather", mybir.AluOpType.bypass, replica_groups=DIR_GROUPS,
            ins=[ci_whh[:]], outs=[co_whh[:]],
        )

        sm = ctx.enter_context(tc.tile_pool(name=f"sm{rep}", bufs=1))
        gx_sb = sm.tile([128, 32, 16], F32)
        nc.sync.dma_start(out=gx_sb, in_=gxT[:])

        # fc weights (fp8, x512) + bias: resident for the whole rep so fc
        # pass-1 tiles can interleave into the scan.
        w8 = sm.tile([128, 16, v_loc], FP8)
        nc.sync.dma_start(out=w8, in_=fcwT[:])
        bias_bc = sm.tile([128, v_loc], F32)
        fcb_ap = fcb[:]
        nc.sync.dma_start(
            out=bias_bc,
            in_=bass.AP(
                tensor=fcb_ap.tensor, offset=fcb_ap.offset, ap=[[0, 128], [1, v_loc]]
            ),
        )

        whh_pool = ctx.enter_context(tc.tile_pool(name=f"whh{rep}", bufs=1))
        hp = ctx.enter_context(tc.tile_pool(name=f"hp{rep}", bufs=2))
        cp = ctx.enter_context(tc.tile_pool(name=f"cp{rep}", bufs=2))
        wk = ctx.enter_context(tc.tile_pool(name=f"wk{rep}", bufs=2))
        psum_scan = ctx.enter_context(
            tc.tile_pool(name=f"ps{rep}", bufs=2, space="PSUM")
        )
        slab = ctx.enter_context(tc.tile_pool(name=f"slab{rep}", bufs=2))
        wkf = ctx.enter_context(tc.tile_pool(name=f"wkf{rep}", bufs=4))
        psum_fc = ctx.enter_context(
            tc.tile_pool(name=f"psf{rep}", bufs=5, space="PSUM")
        )
        p2p = ctx.enter_context(tc.tile_pool(name=f"p2p{rep}", bufs=2))
        spool = ctx.enter_context(tc.tile_pool(name=f"sp{rep}", bufs=1))

        whh_sb = whh_pool.tile([128, 8, 32, 128], BF16)
        for j in range(8):
            nc.sync.dma_start(out=whh_sb[:, j, :, :], in_=co_whh[j])

        s_all = spool.tile([128, n_tt], F32)
        nc.vector.memset(s_all, 0.0)
        nlz = spool.tile([128, n_tt], F32)
        n_vc = v_loc // 500

        h_prev = hp.tile([128, 8, 16], BF16, tag="h")
        nc.sync.dma_start(out=h_prev, in_=h0T[:])
        c_prev = cp.tile([128, 8, 16], F32, tag="c")
        nc.sync.dma_start(out=c_prev, in_=c0T[:])

        def scan_step(t, h_prev, c_prev):
            pg = psum_scan.tile([128, 32, 16], F32, tag="pg")
            gates = wk.tile([128, 32, 16], F32, tag="gates")
            nl = wk.tile([128, 32, 16], F32, tag="nl")
            h_new = hp.tile([128, 8, 16], BF16, tag="h")
            c_new = cp.tile([128, 8, 16], F32, tag="c")
            g4 = lambda tile_, u0: tile_.rearrange(
                "p (s u) b -> p s u b", s=4
            )[:, :, u0 : u0 + 4, :]
            for u0 in (0, 4):
                # each PSUM slice's start->stop accumulation chain must be
                # contiguous on the PE: interleaving groups across slices
                # produces wrong results on hardware.
                for s in range(4):
                    for u in range(u0, u0 + 4):
                        m = s * 8 + u
                        for j in range(8):
                            nc.tensor.matmul(
                                pg[:, m, :], whh_sb[:, j, m, :], h_prev[:, j, :],
                                start=(j == 0), stop=(j == 7),
                            )
                nc.vector.tensor_add(g4(gates, u0), g4(pg, u0), g4(gx_sb, u0))
                gv = gates.rearrange("p (s u) b -> p s u b", s=4)
                nv = nl.rearrange("p (s u) b -> p s u b", s=4)
                nc.scalar.activation(
                    nv[:, 0:3, u0 : u0 + 4, :], gv[:, 0:3, u0 : u0 + 4, :],
                    ACT.Sigmoid,
                )
                nc.scalar.activation(
                    nv[:, 3, u0 : u0 + 4, :], gv[:, 3, u0 : u0 + 4, :], ACT.Tanh
                )
                t1 = wk.tile([128, 4, 16], F32, tag=f"t1_{u0}")
                t2 = wk.tile([128, 4, 16], F32, tag=f"t2_{u0}")
                tnc = wk.tile([128, 4, 16], F32, tag=f"tnc_{u0}")
                cs = slice(u0, u0 + 4)
                nc.vector.tensor_mul(t1, nv[:, 0, cs, :], nv[:, 3, cs, :])
                nc.vector.tensor_mul(t2, nv[:, 1, cs, :], c_prev[:, cs, :])
                nc.vector.tensor_add(c_new[:, cs, :], t1, t2)
                nc.scalar.activation(tnc, c_new[:, cs, :], ACT.Tanh)
                for jj in range(u0, u0 + 4):
                    nc.vector.tensor_mul(
                        h_new[:, jj, :], nv[:, 2, jj, :], tnc[:, jj - u0, :]
                    )
            k, tin = t // spc, t % spc
            nc.sync.dma_start(
                out=outs_c[k][tin // 2][:, tin % 2, :],
                in_=h_new.rearrange("p j b -> p (j b)"),
            )
            if tin == spc - 1:
                nc.gpsimd.collective_compute(
                    "AllGather", mybir.AluOpType.bypass,
                    replica_groups=ALL_GROUP,
                    ins=[outs_c[k][:]], outs=[outsall_c[k][:]],
                )
            return h_new, c_new

        def pass1_tile(tt):
            k, tp = tt // (spc // 2), tt % (spc // 2)
            src = outsall_c[k][:]
            sl = slab.tile([128, 8, 2, 128], BF16, tag="sl")
            stride_s = (spc // 2) * 128 * 2 * 128
            nc.sync.dma_start(
                out=sl,
                in_=bass.AP(
                    tensor=src.tensor,
                    offset=src.offset + tp * (128 * 2 * 128),
                    ap=[[256, 128], [stride_s, 8], [128, 2], [1, 128]],
                ),
            )
            o_t = slab.tile([128, 16, 128], BF16, tag="o_t")
            for s in range(8):
                d_, q_ = s // 4, s % 4
                dst = o_t[:, d_ * 8 : d_ * 8 + 8, :].rearrange(
                    "p j (t b) -> p j t b", t=2
                )[:, :, :, q_ * 16 : q_ * 16 + 16]
                nc.vector.tensor_copy(
                    dst, sl[:, s].rearrange("p t (j b) -> p j t b", j=8)
                )
            for c8 in range(n_vc):
                vs = slice(c8 * 500, (c8 + 1) * 500)
                ps = psum_fc.tile([128, 500], F32)
                for kt in range(16):
                    nc.tensor.matmul(
                        ps, o_t[:, kt, :], w8[:, kt, vs],
                        start=(kt == 0), stop=(kt == 15),
                    )
                lg = wkf.tile([128, 500], BF16, tag="lg")
                nc.vector.scalar_tensor_tensor(
                    lg, ps, 1.0 / W8_SCALE, bias_bc[:, vs],
                    op0=mybir.AluOpType.mult, op1=mybir.AluOpType.add,
                )
                ex = wkf.tile([128, 500], F32, tag="ex")
                part = wkf.tile([128, 1], F32, tag="part")
                nc.scalar.activation(ex, lg, ACT.Exp, accum_out=part)
                nc.vector.tensor_add(
                    s_all[:, tt : tt + 1], s_all[:, tt : tt + 1], part
                )
                nc.sync.dma_start(out=lgst[tt][:, vs], in_=lg)

        def reduce_group(g):
            t0 = g * tpg
            nc.gpsimd.dma_start(out=ccs_in[g][:], in_=s_all[:, t0 : t0 + tpg])
            nc.gpsimd.collective_compute(
                "AllReduce", mybir.AluOpType.add, replica_groups=ALL_GROUP,
                ins=[ccs_in[g][:]], outs=[ccs_out[g][:]],
            )
            s_glob = spool.tile([128, tpg], F32, tag=f"sg{g}")
            nc.gpsimd.dma_start(out=s_glob, in_=ccs_out[g][:])
            nc.scalar.activation(nlz[:, t0 : t0 + tpg], s_glob, ACT.Ln)
            nc.vector.tensor_scalar_mul(
                nlz[:, t0 : t0 + tpg], nlz[:, t0 : t0 + tpg], -1.0
            )

        def pass2_tile(tt):
            for hq in range(4):
                vs = slice(hq * (v_loc // 4), (hq + 1) * (v_loc // 4))
                st = p2p.tile([128, v_loc // 4], BF16, tag="st")
                nc.sync.dma_start(out=st, in_=lgst[tt][:, vs])
                ot = p2p.tile([128, v_loc // 4], F32, tag="ot")
                nc.scalar.activation(ot, st, ACT.Identity, bias=nlz[:, tt : tt + 1])
                nc.sync.dma_start(out=out[tt][:, vs], in_=ot)

        # ---- interleaved schedule ----
        # chunk 0: pure scan. chunks 1..4: one fc pass-1 tile of chunk k-1
        # per 2 steps (fc matmuls fill the PE while the scan's vector tail
        # runs, and keep the PE p-state at full clock). P2(g) interleaves two
        # chunks after its AllReduce. Remainder after the scan.
        NCHv = T // spc
        for t in range(spc):
            h_prev, c_prev = scan_step(t, h_prev, c_prev)
        for k in range(1, NCHv):
            for tin in range(spc):
                t = k * spc + tin
                h_prev, c_prev = scan_step(t, h_prev, c_prev)
                if tin % 2 == 1:
                    pass1_tile((k - 1) * tpg + tin // 2)
                if k >= 3 and tin % 2 == 0 and tin // 2 < tpg:
                    pass2_tile((k - 3) * tpg + tin // 2)
            reduce_group(k - 1)
        # post-scan remainder: last chunk's pass-1 tiles with P2 of groups
        # 2 and 3 interleaved, then the final normalizer group.
        for i, tt in enumerate(range((NCHv - 1) * tpg, NCHv * tpg)):
            pass1_tile(tt)
            pass2_tile(2 * tpg + i)
            pass2_tile(3 * tpg + i)
        reduce_group(NCHv - 1)
        for tt in range((NCHv - 1) * tpg, NCHv * tpg):
            pass2_tile(tt)

        if timing:
            chk_sb = spool.tile([128, 64], F32)
            nc.vector.tensor_copy(chk_sb[:, :n_tt], nlz)
            nc.sync.dma_start(out=chk[:, :n_tt], in_=chk_sb[:, :n_tt])


def prep_v2(x, h0, c0, W_ih, W_hh, b_ih, b_hh, fc_W, fc_b, T):
    """Per-core in_maps. Core c = (d=c//4, q=c%4): direction d, batch rows
    16q..16q+15, W_hh k-chunk pair {2q, 2q+1}, vocab slice c."""
    v_loc = V // NCORES
    # gate reorder i,f,g,o -> i,f,o,g
    perm = np.concatenate(
        [np.arange(0, 2 * H), np.arange(3 * H, 4 * H), np.arange(2 * H, 3 * H)]
    )
    maps = []
    per_dir = {}
    for d in (0, 1):
        whh_p = W_hh[d][perm]
        per_dir[d] = {
            "whh": whh_p.reshape(32, 128, 8, 128).transpose(3, 2, 0, 1),
            # gx = x @ W_ih^T + b_ih + b_hh, permuted: [64, 4096]
            "gx": (x @ W_ih[d].T + b_ih[d] + b_hh[d])[:, perm],
        }
    for c in range(NCORES):
        d, q = c // 4, c % 4
        pd = per_dir[d]
        whh_qv = np.ascontiguousarray(
            pd["whh"][:, 2 * q : 2 * q + 2].transpose(1, 0, 2, 3)
        ).astype(NP_BF16)
        bs = slice(16 * q, 16 * q + 16)
        gxT = np.ascontiguousarray(
            pd["gx"][bs].reshape(16, 32, 128).transpose(2, 1, 0)
        ).astype(np.float32)
        h0T = np.ascontiguousarray(
            h0[d][bs].reshape(16, 8, 128).transpose(2, 1, 0)
        ).astype(NP_BF16)
        c0T = np.ascontiguousarray(
            c0[d][bs].reshape(16, 8, 128).transpose(2, 1, 0)
        ).astype(np.float32)
        wv = fc_W[c * v_loc : (c + 1) * v_loc] * W8_SCALE
        fcwT = np.ascontiguousarray(
            wv.reshape(v_loc, 16, 128).transpose(2, 1, 0)
        ).astype(NP_FP8)
        maps.append(
            {
                "whh_q": whh_qv,
                "gxT": gxT,
                "h0T": h0T,
                "c0T": c0T,
                "fcwT": fcwT,
                "fcb": np.ascontiguousarray(
                    fc_b[c * v_loc : (c + 1) * v_loc].reshape(1, v_loc)
                ).astype(np.float32),
            }
        )
    return maps


def assemble_output(results, T):
    """results[c]["out"] is [n_tt, 128, v_loc], token = t*64 + b."""
    v_loc = V // NCORES
    full = np.concatenate(
        [results[c]["out"].reshape(B * T, v_loc) for c in range(NCORES)], axis=1
    )  # [t*64+b, V]
    return np.ascontiguousarray(full.reshape(T, B, V).transpose(1, 0, 2))


_build_cache = {}


def kernel(x, h0, c0, W_ih, W_hh, b_ih, b_hh, fc_W, fc_b, max_len):
    T = int(max_len)
    x = np.asarray(x, np.float32)
    h0 = np.asarray(h0, np.float32)
    c0 = np.asarray(c0, np.float32)
    W_ih = np.asarray(W_ih, np.float32)
    W_hh = np.asarray(W_hh, np.float32)
    b_ih = np.asarray(b_ih, np.float32)
    b_hh = np.asarray(b_hh, np.float32)
    fc_W = np.asarray(fc_W, np.float32)
    fc_b = np.asarray(fc_b, np.float32)

    if T not in _build_cache:
        _build_cache[T] = build_v2(T)
    nc = _build_cache[T]
    maps = prep_v2(x, h0, c0, W_ih, W_hh, b_ih, b_hh, fc_W, fc_b, T)
    res = run_bass_kernel_spmd(nc, maps, core_ids=list(range(NCORES)))
    return assemble_output([res.results[c] for c in range(NCORES)], T)
